# revision 10
# baseline (speedup 1.0000x reference)
"""3x3 same-conv (NHWC, 32x56x56x128 -> 32x56x56x256) + bias + ReLU on 8 TRN2 cores.

Strategy: data-parallel over batch (4 images/core). Per core, the conv is
9 shifted matmuls accumulated in PSUM with Cin=128 as the contraction dim:
  out[q, cout] = relu( sum_tap XpT[:, q+off_tap]^T @ W[tap] + b )

The padded image is held transposed in SBUF as [cin=128, ~57*58] fp16
where rows use a width-57 layout: each row is 56 data pixels plus ONE
shared zero column serving as both the right pad of row r and the left
pad of row r+1. Anchor groups are exactly 128 wide, so the M=128
stationary operands enable the PE's Fast Weight Load path and LDWEIGHTS
hides behind the 256-col matmul stream.

The input transpose runs entirely off the PE: the HWDGE load performs
the 32x32 block-grid swap ((a r)(b s) -> (b r)(a s)) with DRAM-side
strides, and a DVE stream-transpose (32x32 blocks, with the fp32->fp16
cast folded in) completes the full transpose into the slab. The PE does
nothing but the 900 conv matmuls (plus a short warm-up burst that lifts
the HAM clock gate to 2.4 GHz while the first loads land).

The output is stored anchor-padded ([3200, 256] per image, one big
contiguous DMA per half-image) and de-padded on the host with a numpy
slice, keeping every device-side store large and regular.
"""

import os
from contextlib import ExitStack

import numpy as np

import concourse.bass as bass
import concourse.bacc as bacc
import concourse.mybir as mybir
import concourse.tile as tile
from concourse.bass_utils import run_bass_kernel_spmd

N_CORES = 8
B, H, W, CIN, COUT = 32, 56, 56, 128, 256
BPC = B // N_CORES            # images per core
RS = W + 1                    # row stride in the slab (57: 56 data + 1 shared pad)
A0 = RS + 1                   # slab position of anchor 0 = pixel (0,0)
NG = 25                       # anchor groups per image (25*128 = 3200 >= 56*57)
NANCH = NG * 128              # padded anchors per image
SLABW = 3328                  # slab width: >= A0 + NANCH + RS + 1, padded
NPIX = H * W                  # 3136
RPT = 8                       # image rows per DVE-transpose chunk
TCH = NPIX // (RPT * W)       # 7 transpose chunks per image
GH0 = 13                      # groups in first store half (12 in second)
N_WARM = 40                   # PE warm-up matmuls (lift HAM gate during prologue)

TAP_OFFS = [(dh - 1) * RS + (dw - 1) for dh in range(3) for dw in range(3)]
F32 = mybir.dt.float32
F16 = mybir.dt.float16

LAST_RESULTS = None


def _build(with_bias: bool):
    nc = bacc.Bacc("TRN2", target_bir_lowering=False, debug=False)
    x_h = nc.declare_dram_parameter("prev_a", [BPC, H, W, CIN], F32, isOutput=False)
    w_h = nc.declare_dram_parameter("filter_w", [3, 3, CIN, COUT], F32, isOutput=False)
    b_h = nc.declare_dram_parameter("filter_b", [1, 1, 1, COUT], F32, isOutput=False)
    y_h = nc.declare_dram_parameter("out", [BPC, NANCH, COUT], F32, isOutput=True)
    x_ap, w_ap, b_ap, y_ap = x_h.ap(), w_h.ap(), b_h.ap(), y_h.ap()

    with tile.TileContext(nc) as tc, ExitStack() as ctx:
        const_pool = ctx.enter_context(tc.tile_pool(name="const", bufs=1))
        xslab_pool = ctx.enter_context(tc.tile_pool(name="xslab", bufs=1))
        stage_pool = ctx.enter_context(tc.tile_pool(name="stage", bufs=2))
        stage16_pool = ctx.enter_context(tc.tile_pool(name="stage16", bufs=2))
        out_pool = ctx.enter_context(tc.tile_pool(name="outsb", bufs=3))
        psum_mm = ctx.enter_context(
            tc.tile_pool(name="psmm", bufs=7, space=bass.MemorySpace.PSUM)
        )
        psum_wm = ctx.enter_context(
            tc.tile_pool(name="pswm", bufs=1, space=bass.MemorySpace.PSUM)
        )

        # Per-image transposed padded slabs [cin, SLABW] fp16, width-57 rows
        xslabs = []
        for i in range(BPC):
            sl = xslab_pool.tile([CIN, SLABW], F16, tag=f"xs{i}")
            xslabs.append(sl)
            nc.vector.memset(sl[:, 0:A0], 0.0)  # top pad row + row0 left pad
            # shared pad column between consecutive rows (r=0..54)
            mid = sl[:, A0 + W : A0 + W + 55 * RS].rearrange(
                "p (r c) -> p r c", c=RS
            )
            nc.vector.memset(mid[:, :, 0:1], 0.0)
            # bottom pad row + group-tail slop
            nc.vector.memset(sl[:, A0 + 55 * RS + W : SLABW], 0.0)

        # PE warm-up: short matmuls on a dedicated zeroed tile while loads
        # land. They keep the PE busy through the HAM activity window so the
        # real matmuls start at 2.4 GHz instead of 1.2.
        warm_src = const_pool.tile([128, 192], F16, tag="warmsrc")
        nc.gpsimd.memset(warm_src[:], 0.0)
        ps_warm = psum_wm.tile([128, 64], F32, tag="warm")
        for _ in range(N_WARM):
            nc.tensor.matmul(
                ps_warm[:], warm_src[:, 0:128], warm_src[:, 128:192],
                start=True, stop=True,
            )

        # Weights: [3,3,128,256] -> SBUF [cin=128, tap*256], fp16 cast in-DMA
        wslab = const_pool.tile([CIN, 9 * COUT], F16, tag="wslab")
        nc.gpsimd.dma_start(
            out=wslab[:].rearrange("k (t n) -> k t n", t=9),
            in_=w_ap.rearrange("a b k n -> (a b) k n").transpose([1, 0, 2]),
        )

        if with_bias:
            bias_sb = const_pool.tile([1, COUT], F16, tag="bias")
            nc.gpsimd.dma_start(
                out=bias_sb[:], in_=b_ap.rearrange("a b c n -> (a b c) n")
            )
            ones_sb = const_pool.tile([1, 128], F16, tag="ones")
            nc.gpsimd.memset(ones_sb[:], 1.0)

        # Image load: fp32, with the 32-block grid swap done by DRAM-side
        # strides: stage[(b r), (a s)] = X[(a r), (b s)].  Four HWDGE DMAs
        # (one per cin-block b); split=True halves them for faster first-chunk
        # availability in the prologue.  The HW stream-transpose can't change
        # dtype, so an ACT copy casts fp32 -> fp16 (two pieces per image,
        # aligned to transpose-chunk boundaries) before the DVE transpose.
        CAST_SPLIT = 4 * RPT * W  # 1792 = chunks 0-3 / 4-6 boundary (a < 56)
        def emit_load(i, split=False):
            stg32 = stage_pool.tile([CIN, NPIX], F32, tag="stage32")
            stg16 = stage16_pool.tile([CIN, NPIX], F16, tag="stage16")
            src = (
                x_ap[i]
                .rearrange("h w c -> (h w) c")
                .rearrange("(a r) (b s) -> b r a s", r=32, s=32)
            )
            eng = [nc.sync, nc.scalar]
            bounds = [0, 56, NPIX // 32] if split else [0, NPIX // 32]
            for h in range(len(bounds) - 1):
                a0, a1 = bounds[h], bounds[h + 1]
                for b in range(4):
                    dstb = stg32[32 * b : 32 * b + 32, :].rearrange(
                        "r (a s) -> r a s", s=32
                    )
                    eng[(2 * b + h) % 2].dma_start(
                        out=dstb[:, a0:a1, :], in_=src[b, :, a0:a1, :]
                    )
                if split:
                    f0, f1 = 32 * a0, 32 * a1
                    nc.scalar.activation(
                        stg16[:, f0:f1], stg32[:, f0:f1],
                        mybir.ActivationFunctionType.Copy,
                    )
            if not split:
                for f0, f1 in [(0, CAST_SPLIT), (CAST_SPLIT, NPIX)]:
                    nc.scalar.activation(
                        stg16[:, f0:f1], stg32[:, f0:f1],
                        mybir.ActivationFunctionType.Copy,
                    )
            return stg16

        # DVE stream-transpose chunk k (8 image rows): completes the
        # transpose (32x32 within-block) into the padded slab.
        def emit_transpose(i, stg16, k):
            src = stg16[:, k * RPT * W : (k + 1) * RPT * W].rearrange(
                "p (r c) -> p r c", c=W
            )
            dst = (
                xslabs[i][:, A0 + k * RPT * RS : A0 + (k + 1) * RPT * RS]
                .rearrange("p (r c) -> p r c", c=RS)[:, :, 0:W]
            )
            nc.vector.transpose(dst, src)

        def emit_group(i, g, oslab, o0):
            # 128 anchors [128g, 128g+128); anchor m = 57r + c, junk iff c=56
            q0 = A0 + 128 * g
            ps = psum_mm.tile([128, COUT], F32, tag="psmm")
            for t in range(9):
                w0 = q0 + TAP_OFFS[t]
                nc.tensor.matmul(
                    ps[:],
                    xslabs[i][:, w0 : w0 + 128],
                    wslab[:, t * COUT : (t + 1) * COUT],
                    start=(t == 0),
                    stop=(t == 8 and not with_bias),
                )
            if with_bias:
                nc.tensor.matmul(
                    ps[:], ones_sb[:1, :128], bias_sb[:1, :], start=False, stop=True
                )
            nc.vector.tensor_scalar_max(
                oslab[:, (g - o0) * COUT : (g - o0 + 1) * COUT], ps[:], 0.0
            )

        def emit_store(i, oslab, g0, g1):
            dst = y_ap[i].rearrange("(g p) c -> p g c", p=128)[:, g0:g1, :]
            nc.sync.dma_start(
                out=dst, in_=oslab[:].rearrange("p (g c) -> p g c", g=g1 - g0)
            )

        stg_cur = emit_load(0, split=True)
        stg_nxt = emit_load(1) if BPC > 1 else None
        emit_transpose(0, stg_cur, 0)
        done_cur = 1  # transpose chunks emitted for current image
        done_nxt = 0  # transpose chunks emitted for next image
        for i in range(BPC):
            for g0, g1 in [(0, GH0), (GH0, NG)]:
                oslab = out_pool.tile([128, (g1 - g0) * COUT], F32, tag="osb")
                for g in range(g0, g1):
                    # group g's max slab read is position A0+128g+127+58,
                    # i.e. data row (128g+185)//57 -> chunk row//8
                    need = min(TCH, min(55, (128 * g + 185) // RS) // RPT + 1)
                    while done_cur < need:
                        emit_transpose(i, stg_cur, done_cur)
                        done_cur += 1
                    emit_group(i, g, oslab, g0)
                    if done_cur >= TCH and i + 1 < BPC:
                        # spread next image's transposes over remaining groups
                        want = (g + 1) * TCH // NG
                        while done_nxt < min(want, TCH):
                            emit_transpose(i + 1, stg_nxt, done_nxt)
                            done_nxt += 1
                emit_store(i, oslab, g0, g1)
            if i + 1 < BPC:
                while done_nxt < TCH:
                    emit_transpose(i + 1, stg_nxt, done_nxt)
                    done_nxt += 1
                stg_cur, done_cur = stg_nxt, TCH
                done_nxt = 0
                if i + 2 < BPC:
                    stg_nxt = emit_load(i + 2)

    nc.compile()
    return nc


_CACHE = {}


def _get_nc(with_bias: bool):
    if with_bias not in _CACHE:
        _CACHE[with_bias] = _build(with_bias)
    return _CACHE[with_bias]


def kernel(prev_a, filter_w, filter_b):
    global LAST_RESULTS
    prev_a = np.ascontiguousarray(prev_a, dtype=np.float32)
    filter_w = np.ascontiguousarray(filter_w, dtype=np.float32)
    filter_b = np.ascontiguousarray(filter_b, dtype=np.float32).reshape(1, 1, 1, COUT)
    with_bias = bool(np.any(filter_b))
    nc = _get_nc(with_bias)
    in_maps = [
        {
            "prev_a": prev_a[c * BPC : (c + 1) * BPC],
            "filter_w": filter_w,
            "filter_b": filter_b,
        }
        for c in range(N_CORES)
    ]
    trace = os.environ.get("KERNEL_TRACE") == "1"
    res = run_bass_kernel_spmd(nc, in_maps, list(range(N_CORES)), trace=trace)
    LAST_RESULTS = res
    # de-pad: anchor m = 57*r + c, valid iff c < 56
    outs = []
    for c in range(N_CORES):
        ypad = res.results[c]["out"]
        outs.append(
            ypad[:, : H * RS, :].reshape(BPC, H, RS, COUT)[:, :, :W, :]
        )
    return np.ascontiguousarray(np.concatenate(outs, axis=0))
